# revision 2
# baseline (speedup 1.0000x reference)
"""Conv2d 3x3 VALID stride-1 kernel for Trainium2 (Bass/Tile), 8-core SPMD.

x: [32, 128, 112, 112] f32, weight: [256, 128, 3, 3] f32
out: [32, 256, 110, 110] f32

Strategy: implicit GEMM. Cin=128 sits on the SBUF partition dim and is the
matmul contraction axis. For each of the 9 filter taps (kh, kw), a matmul
with lhsT = weight[ci, co_tile] and rhs = x[ci, shifted-window pixels]
accumulates into PSUM (start on tap 0). Output row-chunks of 4 rows
(free dim 440 <= 512 = one PSUM bank) stream through the PE at 1 fp32r
cycle/row. Data-parallel over batch: 4 images per core.
"""

import numpy as np

import concourse.bass as bass
import concourse.mybir as mybir
import concourse.tile as tile
from concourse import bacc
from concourse.bass_utils import run_bass_kernel_spmd

B, CIN, H, W = 32, 128, 112, 112
COUT, KH, KW = 256, 3, 3
OH, OW = H - KH + 1, W - KW + 1  # 110, 110
NCORES = 8
BPC = B // NCORES  # batches per core

F32 = mybir.dt.float32
F32R = mybir.dt.float32r

# Row-chunking of the 110 output rows: free dim = rows*110, must be <= 512
# and >= 256 (fp32r full-rate threshold). 26*4 + 2*3 = 110.
ROW_CHUNKS = [4] * 26 + [3] * 2

_CACHE = {}


def _build_nc():
    nc = bacc.Bacc("TRN2", target_bir_lowering=False, debug=False)

    x_d = nc.dram_tensor("x", [BPC, CIN, H, W], F32, kind="ExternalInput")
    w_d = nc.dram_tensor("w", [CIN, KH * KW, COUT], F32, kind="ExternalInput")
    o_d = nc.dram_tensor("o", [BPC, COUT, OH, OW], F32, kind="ExternalOutput")

    with tile.TileContext(nc) as tc:
        with (
            tc.tile_pool(name="wpool", bufs=1) as wpool,
            tc.tile_pool(name="xpool", bufs=2) as xpool,
            tc.tile_pool(name="opool", bufs=4) as opool,
            tc.tile_pool(name="psum", bufs=8, space="PSUM") as psum,
        ):
            wr = wpool.tile([CIN, KH * KW, COUT], F32R)
            nc.gpsimd.dma_start(wr[:], w_d[:])

            for b in range(BPC):
                xr = xpool.tile([CIN, H, W], F32R, tag="x")
                # Split the 6.4MB image load into 4 chunks so DMA overlaps
                # with compute on earlier rows.
                for c in range(4):
                    nc.gpsimd.dma_start(
                        xr[:, 28 * c : 28 * (c + 1), :],
                        x_d[b, :, 28 * c : 28 * (c + 1), :],
                    )

                for ct in range(2):
                    co0 = ct * 128
                    oh = 0
                    for R in ROW_CHUNKS:
                        ps = psum.tile([128, R, OW], F32, tag="ps")
                        for idx in range(KH * KW):
                            kh, kw = divmod(idx, KW)
                            nc.tensor.matmul(
                                ps[:],
                                wr[:, idx, co0 : co0 + 128],
                                xr[:, oh + kh : oh + kh + R, kw : kw + OW],
                                start=(idx == 0),
                                stop=(idx == KH * KW - 1),
                            )
                        ot = opool.tile([128, R, OW], F32, tag="ot")
                        nc.vector.tensor_copy(ot[:], ps[:])
                        nc.sync.dma_start(
                            o_d[b, co0 : co0 + 128, oh : oh + R, :], ot[:]
                        )
                        oh += R

    nc.compile()
    return nc


def _get_nc():
    if "nc" not in _CACHE:
        _CACHE["nc"] = _build_nc()
    return _CACHE["nc"]


LAST_RESULT = None


def kernel(x, weight, trace=False):
    global LAST_RESULT
    x = np.ascontiguousarray(np.asarray(x, dtype=np.float32))
    weight = np.asarray(weight, dtype=np.float32)
    # [Cout, Cin, kh, kw] -> [Cin, kh*kw, Cout], contiguous
    w_packed = np.ascontiguousarray(
        weight.transpose(1, 2, 3, 0).reshape(CIN, KH * KW, COUT)
    )

    nc = _get_nc()
    in_maps = [
        {"x": x[i * BPC : (i + 1) * BPC], "w": w_packed} for i in range(NCORES)
    ]
    res = run_bass_kernel_spmd(
        nc, in_maps, core_ids=list(range(NCORES)), trace=trace
    )
    LAST_RESULT = res
    out = np.concatenate([r["o"] for r in res.results], axis=0)
    return out
